# revision 1
# baseline (speedup 1.0000x reference)
"""Trainium2 Bass kernel for ChamferLossSplitPID.

Contract: kernel(**inputs) takes the FULL inputs (target/reco [64,512,4] f32,
in_pid/out_pid [64,512] i32) and returns the full output (loss_nonzero,
loss_zero) as float32 scalars, matching reference().

Strategy (8 NeuronCores, data-parallel over batch, 8 batches per core):
  dist^2[n,m] = |t_n|^2 + |r_m|^2 - 2 t.r computed on the PE as a K=16
  split-bf16 matmul (a.b ~ ahi.bhi + ahi.blo + alo.bhi, norm terms split
  hi/lo too; ~1e-5 relative accuracy at full bf16 speed). The "other side"
  points are permuted into 4 pid groups padded to a fixed S=130 columns
  (pad cols produce dist^2 = 2^27, never a min winner). Because sqrt is
  monotone, per-pid mins are taken on dist^2: all 4 matmuls of a (batch,
  row-chunk) land in one 4-bank PSUM tile at a uniform 256-element slot
  stride, so ONE 3D-AP DVE reduce yields both directions x 4 group minima.
  Only the [128, 16x16] minima get relu+sqrt; a host-built 0/1 row mask
  selects the rows of each pid's sum, and partition sums go through a
  GpSimd all-reduce (mid-stream pieces) / PE ones-matmul (final piece).
  Per-pid norm sums run as one 72-row masked multiply+reduce. The tiny
  O(B*pid) epilogue (counts, divisions, empty-group branches, means) runs
  on the host, as does all layout prep (permutation, hi/lo splits, masks).

The emitted IR is input-value-independent (fixed group stride S), so one
SPMD program serves all 8 cores. S is bumped automatically if some pid
group exceeds it (recompile, still correct for any input).

Measured: ~62.6 us on hardware per core (8 cores run concurrently),
relative error vs the fp32 reference ~3e-6.
"""

import sys

sys.path.insert(0, "/opt/trn_rl_repo")

import numpy as np

from concourse import bacc, bass, bass_isa, mybir, tile
from concourse.bass_utils import run_bass_kernel_spmd

B, N, D = 64, 512, 4
NCORES = 8
BL = B // NCORES          # batches per core
P = 128                   # partitions
NCH = N // P              # row chunks per batch
NPID = 4                  # nonzero pid classes
BIG = float(2 ** 27)      # pad-column dist^2 (exact in bf16)
KROWS = 16                # split-bf16 contraction rows
F32 = mybir.dt.float32
BF16 = mybir.dt.bfloat16

_PROGRAM_CACHE = {}


def _build_program(S: int):
    """Emit the SPMD Bass program for group stride S. Value-independent."""
    COLS = NPID * S           # padded columns per batch per direction
    HALF = COLS // 2          # one matmul = 2 pid groups (<=512 psum floats)
    nc = bacc.Bacc(None)

    # lhsT and rhs for one direction share one tensor/DMA so the first
    # consuming Matmult carries a single sync wait (PE LW allows only one).
    d_ab1 = nc.dram_tensor("ab1", [BL, KROWS, N + COLS], BF16, kind="ExternalInput")
    d_ab2 = nc.dram_tensor("ab2", [BL, KROWS, N + COLS], BF16, kind="ExternalInput")
    d_rm = nc.dram_tensor("rmall", [P, 2 * BL * 16], F32, kind="ExternalInput")
    # norm sums in partition-parallel layout: row g*BL+b, g in (p1..p4 of
    # in_pid, p1..p4 of out_pid, p0 of out_pid)
    d_nrm = nc.dram_tensor("normrep", [9 * BL, N], F32, kind="ExternalInput")
    d_msk = nc.dram_tensor("mask72", [9 * BL, N], F32, kind="ExternalInput")
    d_sums = nc.dram_tensor("sums", [1, 2 * BL * NPID], F32, kind="ExternalOutput")
    d_ns = nc.dram_tensor("ns", [9 * BL, 1], F32, kind="ExternalOutput")

    with tile.TileContext(nc) as tc:
        with (
            tc.tile_pool(name="const", bufs=1) as const,
            tc.tile_pool(name="work", bufs=2) as work,
            tc.tile_pool(name="psum", bufs=2, space=bass.MemorySpace.PSUM) as psum,
        ):
            # one tile + one DMA per (dir, batch): matmuls for batch b start
            # as soon as its slice lands. dir-0 loads issue from the Sync
            # HWDGE, dir-1 from the Activation HWDGE (parallel issue).
            d_ab = [d_ab1, d_ab2]
            tAB = [[const.tile([KROWS, N + COLS], BF16, tag=f"ab{d}_{b}", name=f"tAB{d}_{b}")
                    for b in range(BL)] for d in range(2)]
            for b in range(BL):
                for d in range(2):
                    eng = nc.sync if d == 0 else nc.scalar
                    eng.dma_start(tAB[d][b][:], d_ab[d][b])
            tRM = const.tile([P, 2 * BL, NCH, NPID], F32, tag="rm")
            nc.sync.dma_start(tRM[:], d_rm[:].rearrange("p (a c q) -> p a c q", q=NPID, c=NCH))
            tNRM = const.tile([9 * BL, N], F32, tag="nrm")
            tMSK = const.tile([9 * BL, N], F32, tag="msk")
            nc.scalar.dma_start(tNRM[:], d_nrm[:])
            nc.scalar.dma_start(tMSK[:], d_msk[:])
            tONE = const.tile([P, 1], F32, tag="one")
            nc.vector.memset(tONE[:], 1.0)

            # per-pid norm sums, partition-parallel — emitted early so DVE
            # slots them into the pipeline ramp instead of the drain tail
            tNS = work.tile([9 * BL, 1], F32, tag="nsout")
            tmp72 = work.tile([9 * BL, N], F32, tag="tmp72")
            nc.vector.tensor_tensor(tmp72[:], tNRM[:], tMSK[:], op=mybir.AluOpType.mult)
            nc.vector.tensor_reduce(
                tNS[:], tmp72[:], axis=mybir.AxisListType.X, op=mybir.AluOpType.add)
            nc.sync.dma_start(d_ns[:], tNS[:])

            # minima of dist^2: [128, (b,dir), chunk, pid] (pid contiguous;
            # (b,dir) batch-major so each batch-half is a contiguous slice)
            tMS = const.tile([P, 2 * BL, NCH, NPID], F32, tag="ms")
            tSQ = const.tile([P, 2 * BL * NPID * NCH], F32, tag="sq")
            tMK = const.tile([P, 2 * BL * NPID * NCH], F32, tag="mk")
            tPR = const.tile([P, 2 * BL, NCH, NPID], F32, tag="pr")
            tSF = const.tile([1, 2 * BL * NPID], F32, tag="sf")

            NQ = 4  # tail pieces (2 batches each)

            def tail_half(h):
                # relu -> sqrt -> row-mask -> partition-sum -> chunk-sum for
                # dbs [h*4, h*4+4). Pieces 0-2 sum on idle GpSimd (mid-stream,
                # PSUM fully busy); the last piece uses a PE ones-matmul into
                # a now idle dist-pool PSUM slot (GpSimd is too slow there).
                w = 2 * BL * NPID * NCH // NQ  # 64 cols per piece
                sl = slice(h * w, (h + 1) * w)
                flat = tMS[:].rearrange("p a c q -> p (a c q)")[:, sl]
                nc.vector.tensor_scalar_max(flat, flat, 0.0)
                nc.scalar.activation(tSQ[:, sl], flat, mybir.ActivationFunctionType.Sqrt)
                nc.vector.tensor_tensor(
                    tMK[:, sl], tSQ[:, sl],
                    tRM[:].rearrange("p a c q -> p (a c q)")[:, sl],
                    op=mybir.AluOpType.mult,
                )
                hdb = 2 * BL // NQ  # dbs per piece
                if h < NQ - 1:
                    nc.gpsimd.partition_all_reduce(
                        tPR[:].rearrange("p a c q -> p (a c q)")[:, sl],
                        tMK[:, sl], P, bass_isa.ReduceOp.add,
                    )
                    srcrow = tPR[0:1, h * hdb:(h + 1) * hdb].rearrange("o a c q -> o a q c")
                else:
                    prow = psum.tile([1, hdb, NCH, NPID], F32, tag="dist", name="prow")
                    nc.tensor.matmul(
                        prow[:].rearrange("o a c q -> o (a c q)"),
                        tONE[:],
                        tMK[:, sl],
                        start=True,
                        stop=True,
                    )
                    srcrow = prow[:].rearrange("o a c q -> o a q c")
                nc.vector.tensor_reduce(
                    tSF[:].rearrange("o (a q) -> o a q", q=NPID)[:, h * hdb:(h + 1) * hdb, :],
                    srcrow,
                    axis=mybir.AxisListType.X,
                    op=mybir.AluOpType.add,
                )

            HB = BL  # dbs per half (4 batches x 2 dirs)
            for b in range(BL):
                for c in range(NCH):
                    if S <= 256:
                        # fast path: one 4-bank tile holds all 4 matmuls of
                        # (b,c); slot s = dr*4+j*2+g at uniform 256-elem
                        # stride, so ONE 3D-AP reduce covers both dirs x 4
                        # groups
                        pt = psum.tile([P, 8, 256], F32, tag="dist")
                        for dr in range(2):
                            for j in range(2):
                                nc.tensor.matmul(
                                    pt[:, dr * 4 + 2 * j : dr * 4 + 2 * j + 2, 0:S],
                                    tAB[dr][b][:, c * P : (c + 1) * P],
                                    tAB[dr][b][:, N + j * HALF : N + (j + 1) * HALF],
                                    start=True,
                                    stop=True,
                                )
                        nc.vector.tensor_reduce(
                            tMS[:, 2 * b : 2 * b + 2, c, :],
                            pt[:, :, 0:S],
                            axis=mybir.AxisListType.X,
                            op=mybir.AluOpType.min,
                        )
                    else:
                        # big-group fallback (S in (256, 512]): one matmul
                        # per pid group, 512-elem slots, per-direction tiles
                        for dr in range(2):
                            pt = psum.tile([P, 4, 512], F32, tag="dist")
                            for g in range(NPID):
                                nc.tensor.matmul(
                                    pt[:, g, 0:S],
                                    tAB[dr][b][:, c * P : (c + 1) * P],
                                    tAB[dr][b][:, N + g * S : N + (g + 1) * S],
                                    start=True,
                                    stop=True,
                                )
                            nc.vector.tensor_reduce(
                                tMS[:, 2 * b + dr, c, :],
                                pt[:, :, 0:S],
                                axis=mybir.AxisListType.X,
                                op=mybir.AluOpType.min,
                            )
                if b % 2 == 1 and b < BL - 1:
                    q = b // 2
                    tail_half(q)
                    w2 = 2 * BL * NPID // NQ
                    nc.sync.dma_start(
                        d_sums[:, q * w2 : (q + 1) * w2], tSF[:, q * w2 : (q + 1) * w2])
            tail_half(NQ - 1)
            nc.sync.dma_start(
                d_sums[:, (NQ - 1) * 2 * BL * NPID // NQ :],
                tSF[:, (NQ - 1) * 2 * BL * NPID // NQ :])



    nc.compile()
    return nc


def _get_program(S: int):
    if S not in _PROGRAM_CACHE:
        _PROGRAM_CACHE[S] = _build_program(S)
    return _PROGRAM_CACHE[S]


def _prep_inputs(target, reco, in_pid, out_pid, S):
    """Build per-core input maps. All heavy compute stays on device; this is
    O(B*N) metadata/layout prep (permutation, norms, masks, padding)."""
    COLS = NPID * S
    t = np.ascontiguousarray(np.asarray(target, dtype=np.float32))
    r = np.ascontiguousarray(np.asarray(reco, dtype=np.float32))
    ip = np.asarray(in_pid)
    op = np.asarray(out_pid)

    import ml_dtypes

    def split16(x):
        hi = x.astype(ml_dtypes.bfloat16).astype(np.float32)
        lo = (x - hi).astype(ml_dtypes.bfloat16).astype(np.float32)
        return hi, lo

    nt2 = (t * t).sum(-1)                      # [B,N]
    nr2 = (r * r).sum(-1)
    ones = np.ones((B, 1, N), np.float32)
    # split-bf16: a.b ~= ahi.bhi + ahi.blo + alo.bhi (lo.lo dropped, ~2^-16 rel)
    # lhsT rows: [(-2x)hi x4, (-2x)hi x4, (-2x)lo x4, |x|2hi, |x|2lo, 1, 1]
    # rhs rows:  [ yhi x4,     ylo x4,     yhi x4,    1,      1, |y|2hi, |y|2lo]
    def build_lhs(x, x2):
        m2hi, m2lo = split16(-2.0 * x.transpose(0, 2, 1))   # [B,4,N]
        x2hi, x2lo = split16(x2[:, None, :])                # [B,1,N]
        return np.concatenate(
            [m2hi, m2hi, m2lo, x2hi, x2lo, ones, ones], axis=1)  # [B,16,N]

    def build_rhs(x, x2, pid):
        rhs = np.zeros((B, KROWS, COLS), np.float32)
        rhs[:, 14, :] = BIG
        xhi, xlo = split16(x)                               # [B,N,4]
        y2hi, y2lo = split16(x2)                            # [B,N]
        for b in range(B):
            for p in range(1, 5):
                idx = np.nonzero(pid[b] == p)[0]
                k = len(idx)
                if k == 0:
                    continue
                c0 = (p - 1) * S
                rhs[b, 0:4, c0 : c0 + k] = xhi[b, idx].T
                rhs[b, 4:8, c0 : c0 + k] = xlo[b, idx].T
                rhs[b, 8:12, c0 : c0 + k] = xhi[b, idx].T
                rhs[b, 12:14, c0 : c0 + k] = 1.0
                rhs[b, 14, c0 : c0 + k] = y2hi[b, idx]
                rhs[b, 15, c0 : c0 + k] = y2lo[b, idx]
        return rhs

    lhs1 = build_lhs(t, nt2)
    lhs2 = build_lhs(r, nr2)
    rhs1 = build_rhs(r, nr2, op)   # dir0: rows=targets, cols=reco groups
    rhs2 = build_rhs(t, nt2, ip)   # dir1: rows=recos,  cols=target groups

    # row masks [B, 128, 2, 16]: (dir, b) -> col (pid-1)*4 + chunk
    pgrid = np.arange(1, 5)
    ohx = (ip.reshape(B, NCH, P)[:, :, :, None] == pgrid).astype(np.float32)  # [B,c,i,p]
    ohy = (op.reshape(B, NCH, P)[:, :, :, None] == pgrid).astype(np.float32)
    # -> [B, i(128), c, p]
    rm1 = ohx.transpose(0, 2, 1, 3)
    rm2 = ohy.transpose(0, 2, 1, 3)

    normt = np.sqrt(nt2).astype(np.float32)
    normr = np.sqrt(nr2).astype(np.float32)
    # 72-row layout: row g*BL+b; g 0-3: normt & in_pid==g+1;
    # g 4-7: normr & out_pid==g-3; g 8: normr & out_pid==0
    grp_norm = [normt] * 4 + [normr] * 5
    grp_mask = [(ip == p) for p in (1, 2, 3, 4)] + [(op == p) for p in (1, 2, 3, 4, 0)]

    in_maps = []
    for ci in range(NCORES):
        s = slice(ci * BL, (ci + 1) * BL)
        rm = np.zeros((P, 2 * BL, NCH, NPID), np.float32)
        rm[:, 0::2] = rm1[s].transpose(1, 0, 2, 3)
        rm[:, 1::2] = rm2[s].transpose(1, 0, 2, 3)
        ab1 = np.concatenate([lhs1[s], rhs1[s]], axis=2)  # [BL,16,N+COLS]
        ab2 = np.concatenate([lhs2[s], rhs2[s]], axis=2)
        normrep = np.concatenate([g[s] for g in grp_norm], axis=0)          # [72,N]
        mask72 = np.concatenate([g[s].astype(np.float32) for g in grp_mask], axis=0)
        in_maps.append({
            "ab1": np.ascontiguousarray(ab1.astype(ml_dtypes.bfloat16)),
            "ab2": np.ascontiguousarray(ab2.astype(ml_dtypes.bfloat16)),
            "rmall": np.ascontiguousarray(rm.reshape(P, 2 * BL * 16)),
            "normrep": np.ascontiguousarray(normrep),
            "mask72": np.ascontiguousarray(mask72),
        })
    return in_maps


def _epilogue(sums_all, ns_all, in_pid, out_pid):
    """Tiny O(B*pid) final combination, mirrors reference()'s branch logic."""
    ip = np.asarray(in_pid)
    op = np.asarray(out_pid)
    sum_xy = np.zeros((B, 5))
    sum_yx = np.zeros((B, 5))
    only_x = np.zeros((B, 5))
    only_y = np.zeros((B, 5))
    zerosum = np.zeros(B)
    for ci in range(NCORES):
        srow = sums_all[ci].reshape(BL, 2, NPID)
        nsrow = ns_all[ci]
        for lb in range(BL):
            b = ci * BL + lb
            sum_xy[b, 1:5] = srow[lb, 0]
            sum_yx[b, 1:5] = srow[lb, 1]
            ns72 = nsrow.reshape(9, BL)
            only_x[b, 1:5] = ns72[0:4, lb]
            only_y[b, 1:5] = ns72[4:8, lb]
            zerosum[b] = ns72[8, lb]

    cx = np.stack([(ip == p).sum(1) for p in range(5)], 1)  # [B,5]
    cy = np.stack([(op == p).sum(1) for p in range(5)], 1)

    loss_nonzero = np.float32(0.0)
    for p in range(1, 5):
        both = 0.5 * (sum_xy[:, p] / np.maximum(1, cy[:, p])
                      + sum_yx[:, p] / np.maximum(1, cx[:, p]))
        ox = only_x[:, p] / np.maximum(1, cx[:, p])
        oy = only_y[:, p] / np.maximum(1, cy[:, p])
        per_b = np.where(cy[:, p] == 0, ox, np.where(cx[:, p] == 0, oy, both))
        loss_nonzero = loss_nonzero + np.float32(per_b.mean())
    loss_zero = np.float32((zerosum / np.maximum(1, cy[:, 0])).mean())
    return np.float32(loss_nonzero), np.float32(loss_zero)


def kernel(target, reco, in_pid, out_pid):
    ip = np.asarray(in_pid)
    op = np.asarray(out_pid)
    # fixed group stride; bump (recompile) only if a pid group overflows it
    max_grp = 0
    for pid in (ip, op):
        for p in range(1, 5):
            max_grp = max(max_grp, int((pid == p).sum(1).max()))
    S = 130
    while S < max_grp:
        S += 8
    S = min(S, 512)  # a pid group can never exceed N=512

    nc = _get_program(S)
    in_maps = _prep_inputs(target, reco, ip, op, S)
    res = run_bass_kernel_spmd(nc, in_maps, list(range(NCORES)))
    sums_all = [res.results[ci]["sums"] for ci in range(NCORES)]
    ns_all = [res.results[ci]["ns"] for ci in range(NCORES)]
    return _epilogue(sums_all, ns_all, ip, op)



# revision 5
# speedup vs baseline: 1.8113x; 1.8113x over previous
"""Trainium2 Bass kernel for ChamferLossSplitPID (block-diagonal rewrite).

Contract: kernel(**inputs) takes the FULL inputs (target/reco [64,512,4] f32,
in_pid/out_pid [64,512] i32) and returns the full output (loss_nonzero,
loss_zero) as float32 scalars, matching reference().

Key observation vs the v0 kernel: the reference only ever takes min distances
between SAME-pid groups (for pid p: targets with in_pid==p vs recos with
out_pid==p).  So the needed distance matrix is block-diagonal.  Both sides are
permuted into 4 pid groups of <=128 points (stride 128); per (batch, dir) ONE
K=64 matmul computes all 4 diagonal blocks at once: the stationary holds the
4 groups of side A banded along K (16 formula rows per group), and the moving
tensor holds side B's columns with each group's 16 rows placed in its own
K-band (zeros elsewhere), so column c of group g only contracts against band
g.  dist^2 = |x|^2+|y|^2-2x.y is built from a split-bf16 expansion (hi/lo,
~1e-5 rel accuracy).  Pad columns carry 2^27 in the norm row (never win the
min); pad rows produce 0 and are masked out of the sums.

Per (batch, dir) the [128, 4x128] PSUM block is min-reduced per group.  The
PSUM readout is the bottleneck (DVE fp32-from-PSUM runs 1 elem/cycle/lane), so
the 8 (batch,dir) pairs are split between two engines: DVE min-reduces some
pairs straight from PSUM while the Activation engine evicts the others to SBUF
as bf16 where DVE re-reduces them at 4x.  The [128, 64] minima get
sqrt(x+1e-3) (bias instead of relu: split-bf16 can go slightly negative),
a row-validity mask, and a GpSimd partition-sum.  Norm sums for the pid-0 /
empty-group terms run as one 72-row reduce of host-precomputed norm*mask
products.  The tiny O(B*pid) epilogue (counts, divisions, empty-group
branches, means) runs on the host; groups that overflow 128 members (~0.2% of
group instances) are patched exactly on the host.

One fixed SPMD program serves all 8 cores (data-parallel over batch, 8
batches/core); the emitted IR is input-value-independent.
"""

import sys

sys.path.insert(0, "/opt/trn_rl_repo")

import numpy as np

from concourse import bacc, bass, bass_isa, mybir, tile
from concourse.bass_utils import run_bass_kernel_spmd

B, N, D = 64, 512, 4
NCORES = 8
BL = B // NCORES          # batches per core
P = 128                   # partitions
NPID = 4                  # nonzero pid classes
SC = 128                  # group stride (rows and cols)
NU = 2 * BL               # units per core: u = local_batch*2 + dir
KROWS = 16                # split-bf16 contraction rows per group
BIG = float(2 ** 27)      # pad-column dist^2 (exact in bf16)
SQRT_BIAS = 1e-3          # sqrt(x + bias): guards split-bf16 negatives
F32 = mybir.dt.float32
BF16 = mybir.dt.bfloat16

NBAND = 4                 # pid groups banded per matmul (1, 2 or 4)
NMM = NPID // NBAND       # matmuls per unit
# per PSUM pair (2 units): 'D' = DVE min-reduce straight from PSUM,
# 'A' = ACT evicts to SBUF bf16, DVE re-reduces at 4x
PAIR_KIND = "DAADADAA"
# unit ranges covered by each input-DMA chunk (staged for early compute start)
ST_CHUNKS = [(0, 4), (4, 16)]
RHS_CHUNKS = [(0, 1), (1, 2), (2, 4), (4, 8), (8, 16)]

_PROGRAM_CACHE = {}


def _chunk_of(chunks, u):
    for i, (u0, u1) in enumerate(chunks):
        if u0 <= u < u1:
            return i, u0
    raise ValueError(u)


def _build_program():
    nc = bacc.Bacc(None)
    d_st = nc.dram_tensor("st", [4 * KROWS, NU * SC], BF16, kind="ExternalInput")
    d_rhs = nc.dram_tensor("rhs", [NBAND * KROWS, NU * NPID * SC], BF16,
                           kind="ExternalInput")
    d_msk = nc.dram_tensor("msk", [P, NU * NPID], F32, kind="ExternalInput")
    d_nm = nc.dram_tensor("nm", [9 * BL, N], BF16, kind="ExternalInput")
    d_sums = nc.dram_tensor("sums", [1, NU * NPID], F32, kind="ExternalOutput")
    d_ns = nc.dram_tensor("ns", [9 * BL, 1], F32, kind="ExternalOutput")

    UC = NPID * SC            # columns per unit (512)
    with tile.TileContext(nc) as tc:
        with (
            tc.tile_pool(name="const", bufs=1) as const,
            tc.tile_pool(name="psum", bufs=4, space=bass.MemorySpace.PSUM) as psum,
        ):
            tST = [const.tile([4 * KROWS, (u1 - u0) * SC], BF16, tag=f"st{i}", name=f"tST{i}")
                   for i, (u0, u1) in enumerate(ST_CHUNKS)]
            tRHS = [const.tile([NBAND * KROWS, (u1 - u0) * UC], BF16, tag=f"rhs{i}", name=f"tRHS{i}")
                    for i, (u0, u1) in enumerate(RHS_CHUNKS)]
            tMSK = const.tile([P, NU * NPID], F32, tag="msk")
            tNM = const.tile([9 * BL, N], BF16, tag="nm")
            tMS = const.tile([P, NU, NPID], BF16, tag="ms")
            tEV = {i: const.tile([P, 2, UC], BF16, tag=f"ev{i}", name=f"tEV{i}")
                   for i, k in enumerate(PAIR_KIND) if k == "A"}
            tSQ = const.tile([P, NU * NPID], F32, tag="sq")
            tMK = const.tile([P, NU * NPID], F32, tag="mk")
            tPR = const.tile([P, NU * NPID], F32, tag="pr")
            tNS = const.tile([9 * BL, 1], F32, tag="ns")
            tD0 = const.tile([1, 1], F32, tag="d0")
            tD1 = const.tile([1, 1], F32, tag="d1")
            tBIAS = const.tile([P, 1], F32, tag="bias")

            # input DMAs; rhs chunks on the SP queue, st chunks on ACT's,
            # small tensors from DVE's (each queue's DGE-config time is what
            # staggers the chunks, hence small-first)
            for i, (u0, u1) in enumerate(RHS_CHUNKS):
                nc.sync.dma_start(tRHS[i][:], d_rhs[:, u0 * UC:u1 * UC])
            for i, (u0, u1) in enumerate(ST_CHUNKS):
                nc.scalar.dma_start(tST[i][:], d_st[:, u0 * SC:u1 * SC])
            nc.vector.memset(tD0[:], 0.25)
            nc.vector.memset(tBIAS[:], SQRT_BIAS)
            nc.gpsimd.dma_start(tNM[:], d_nm[:])
            nc.gpsimd.dma_start(tMSK[:], d_msk[:])
            # preload the sqrt activation-table set (contains copy too) so the
            # tail sqrt doesn't pay the table load
            nc.scalar.activation(tD1[:], tD0[:], mybir.ActivationFunctionType.Sqrt)

            # norm*mask sums (pid-0 loss + empty-group terms), early on DVE
            nc.vector.tensor_reduce(
                tNS[:], tNM[:], axis=mybir.AxisListType.X, op=mybir.AluOpType.add)
            nc.gpsimd.dma_start(d_ns[:], tNS[:])

            for pair in range(NU // 2):
                pt = psum.tile([P, 2, UC], F32, tag="dist")
                for k in range(2):
                    u = pair * 2 + k
                    si, su0 = _chunk_of(ST_CHUNKS, u)
                    ri, ru0 = _chunk_of(RHS_CHUNKS, u)
                    soff = (u - su0) * SC
                    roff = (u - ru0) * UC
                    for j in range(NMM):
                        nc.tensor.matmul(
                            pt[:, k, j * NBAND * SC:(j + 1) * NBAND * SC],
                            tST[si][KROWS * j * NBAND:KROWS * (j + 1) * NBAND,
                                    soff:soff + SC],
                            tRHS[ri][:, roff + j * NBAND * SC:
                                     roff + (j + 1) * NBAND * SC],
                            start=True,
                            stop=True,
                        )
                if PAIR_KIND[pair] == "D":
                    nc.vector.tensor_reduce(
                        tMS[:, pair * 2:pair * 2 + 2, :],
                        pt[:].rearrange("p k (g c) -> p k g c", g=NPID),
                        axis=mybir.AxisListType.X,
                        op=mybir.AluOpType.min,
                    )
                else:
                    nc.scalar.copy(tEV[pair][:], pt[:])
                    nc.vector.tensor_reduce(
                        tMS[:, pair * 2:pair * 2 + 2, :],
                        tEV[pair][:].rearrange("p k (g c) -> p k g c", g=NPID),
                        axis=mybir.AxisListType.X,
                        op=mybir.AluOpType.min,
                    )

            # tail: sqrt -> row mask -> partition sum -> out
            nc.scalar.activation(
                tSQ[:], tMS[:].rearrange("p u g -> p (u g)"),
                mybir.ActivationFunctionType.Sqrt, bias=tBIAS[:])
            nc.vector.tensor_tensor(tMK[:], tSQ[:], tMSK[:], op=mybir.AluOpType.mult)
            nc.gpsimd.partition_all_reduce(tPR[:], tMK[:], P, bass_isa.ReduceOp.add)
            nc.sync.dma_start(d_sums[:], tPR[0:1, :])

    nc.compile()
    return nc


def _get_program():
    if "p" not in _PROGRAM_CACHE:
        _PROGRAM_CACHE["p"] = _build_program()
    return _PROGRAM_CACHE["p"]


def _group_meta(pid):
    """Per (batch, pid 1..4): member indices padded to SC, validity, counts."""
    order = np.argsort(pid, axis=1, kind="stable")          # [B, N]
    counts = np.stack([(pid == p).sum(1) for p in range(5)], 1)  # [B, 5]
    start = np.zeros((B, 5), np.int64)
    start[:, 1:] = np.cumsum(counts, 1)[:, :-1]
    j = np.arange(SC)
    pos = start[:, 1:5, None] + j                           # [B, 4, SC]
    valid = (j < np.minimum(counts[:, 1:5, None], SC)).astype(np.float32)
    pos = np.minimum(pos, N - 1)
    idx = order[np.arange(B)[:, None, None], pos]           # [B, 4, SC]
    return idx, valid, counts


def _split16(x):
    import ml_dtypes
    hi = x.astype(ml_dtypes.bfloat16).astype(np.float32)
    lo = (x - hi).astype(ml_dtypes.bfloat16).astype(np.float32)
    return hi, lo


def _build_side(pts, pid):
    """L (lhs-role) / R (rhs-role) [B, 4, 16, SC] structure tensors + meta."""
    idx, valid, counts = _group_meta(pid)
    g = pts[np.arange(B)[:, None, None], idx] * valid[..., None]  # [B,4,SC,D]
    x2 = (g * g).sum(-1)                                    # [B, 4, SC]
    m2hi, m2lo = _split16(-2.0 * g)
    x2hi, x2lo = _split16(x2)
    yhi, ylo = _split16(g)
    L = np.zeros((B, NPID, KROWS, SC), np.float32)
    R = np.zeros((B, NPID, KROWS, SC), np.float32)
    mt = (0, 1, 3, 2)
    L[:, :, 0:4] = m2hi.transpose(mt)
    L[:, :, 4:8] = m2hi.transpose(mt)
    L[:, :, 8:12] = m2lo.transpose(mt)
    L[:, :, 12] = x2hi
    L[:, :, 13] = x2lo
    L[:, :, 14] = valid
    L[:, :, 15] = valid
    R[:, :, 0:4] = yhi.transpose(mt)
    R[:, :, 4:8] = ylo.transpose(mt)
    R[:, :, 8:12] = yhi.transpose(mt)
    R[:, :, 12] = valid
    R[:, :, 13] = valid
    R[:, :, 14] = x2hi * valid + BIG * (1.0 - valid)
    R[:, :, 15] = x2lo * valid
    return L, R, valid, counts


def _prep_inputs(target, reco, in_pid, out_pid):
    """Build per-core input maps. O(B*N) layout prep only (permutation,
    bf16 hi/lo splits, norms, masks); all heavy compute stays on device."""
    import ml_dtypes
    t = np.ascontiguousarray(np.asarray(target, dtype=np.float32))
    r = np.ascontiguousarray(np.asarray(reco, dtype=np.float32))
    ip = np.asarray(in_pid)
    op = np.asarray(out_pid)

    Lt, Rt, vt, _ = _build_side(t, ip)
    Lr, Rr, vr, _ = _build_side(r, op)

    normt = np.sqrt((t * t).sum(-1)).astype(np.float32)     # [B, N]
    normr = np.sqrt((r * r).sum(-1)).astype(np.float32)
    grp_prod = ([normt * (ip == p) for p in (1, 2, 3, 4)]
                + [normr * (op == p) for p in (1, 2, 3, 4, 0)])

    UC = NPID * SC
    in_maps = []
    for ci in range(NCORES):
        st = np.zeros((4 * KROWS, NU * SC), np.float32)
        rhs = np.zeros((NBAND * KROWS, NU * UC), np.float32)
        msk = np.zeros((P, NU * NPID), np.float32)
        for lb in range(BL):
            b = ci * BL + lb
            for dr in range(2):
                u = lb * 2 + dr
                LA = Lt if dr == 0 else Lr
                RB = Rr if dr == 0 else Rt
                vA = vt if dr == 0 else vr
                for gi in range(NPID):
                    st[KROWS * gi:KROWS * (gi + 1), u * SC:(u + 1) * SC] = LA[b, gi]
                    jj, kk = gi // NBAND, gi % NBAND
                    c0 = u * UC + jj * NBAND * SC + kk * SC
                    rhs[KROWS * kk:KROWS * (kk + 1), c0:c0 + SC] = RB[b, gi]
                    msk[:, u * NPID + gi] = vA[b, gi]
        nm = np.concatenate([gp[ci * BL:(ci + 1) * BL] for gp in grp_prod], 0)
        in_maps.append({
            "st": np.ascontiguousarray(st.astype(ml_dtypes.bfloat16)),
            "rhs": np.ascontiguousarray(rhs.astype(ml_dtypes.bfloat16)),
            "msk": np.ascontiguousarray(msk),
            "nm": np.ascontiguousarray(nm.astype(ml_dtypes.bfloat16)),
        })
    return in_maps


def _epilogue(sums_all, ns_all, target, reco, in_pid, out_pid):
    """Tiny O(B*pid) final combination mirroring reference()'s branch logic,
    plus exact host recompute for rare >SC-member groups."""
    ip = np.asarray(in_pid)
    op = np.asarray(out_pid)
    t = np.asarray(target, dtype=np.float32)
    r = np.asarray(reco, dtype=np.float32)
    sum_xy = np.zeros((B, 5))
    sum_yx = np.zeros((B, 5))
    only_x = np.zeros((B, 5))
    only_y = np.zeros((B, 5))
    zerosum = np.zeros(B)
    for ci in range(NCORES):
        srow = np.asarray(sums_all[ci]).reshape(BL, 2, NPID)
        ns72 = np.asarray(ns_all[ci]).reshape(9, BL)
        for lb in range(BL):
            b = ci * BL + lb
            sum_xy[b, 1:5] = srow[lb, 0]
            sum_yx[b, 1:5] = srow[lb, 1]
            only_x[b, 1:5] = ns72[0:4, lb]
            only_y[b, 1:5] = ns72[4:8, lb]
            zerosum[b] = ns72[8, lb]

    cx = np.stack([(ip == p).sum(1) for p in range(5)], 1)  # [B, 5]
    cy = np.stack([(op == p).sum(1) for p in range(5)], 1)

    # exact patch for groups with >SC members (device saw only the first SC)
    for b, p in zip(*np.where((cx[:, 1:] > SC) | (cy[:, 1:] > SC))):
        p = p + 1
        tx = t[b][ip[b] == p]
        ry = r[b][op[b] == p]
        if len(tx) and len(ry):
            dd = np.sqrt(((tx[:, None, :] - ry[None, :, :]) ** 2).sum(-1))
            sum_xy[b, p] = dd.min(1).sum()
            sum_yx[b, p] = dd.min(0).sum()

    loss_nonzero = np.float32(0.0)
    for p in range(1, 5):
        both = 0.5 * (sum_xy[:, p] / np.maximum(1, cy[:, p])
                      + sum_yx[:, p] / np.maximum(1, cx[:, p]))
        ox = only_x[:, p] / np.maximum(1, cx[:, p])
        oy = only_y[:, p] / np.maximum(1, cy[:, p])
        per_b = np.where(cy[:, p] == 0, ox, np.where(cx[:, p] == 0, oy, both))
        loss_nonzero = loss_nonzero + np.float32(per_b.mean())
    loss_zero = np.float32((zerosum / np.maximum(1, cy[:, 0])).mean())
    return np.float32(loss_nonzero), np.float32(loss_zero)


def kernel(target, reco, in_pid, out_pid):
    nc = _get_program()
    in_maps = _prep_inputs(target, reco, in_pid, out_pid)
    res = run_bass_kernel_spmd(nc, in_maps, list(range(NCORES)))
    sums_all = [res.results[ci]["sums"] for ci in range(NCORES)]
    ns_all = [res.results[ci]["ns"] for ci in range(NCORES)]
    return _epilogue(sums_all, ns_all, target, reco, in_pid, out_pid)
